# revision 18
# baseline (speedup 1.0000x reference)
"""GCN layer kernel for 8 Trainium2 NeuronCores.

Computes out = relu((A @ H) @ W) where A is a sparse COO matrix given by
(a_rows, a_cols, a_vals); bias b is pinned to zeros by the problem spec.

Strategy (SPMD, one program on 8 cores, per-core data), v2e:
 - Destination rows are LPT-packed on the host into 3136 bins (32 rows,
   <=512 edges each); each core gets 392 bins as its static 32-row dest
   windows, heaviest-first so chunk counts align across cores (the
   per-window chunk count is maxed over cores for a uniform program).
 - Host pre-stages per-slot operands so the device only streams one
   contiguous array (no dma_gather, no Q7 descriptor generation): per
   chunk c, GS[:, c*96:(c+1)*96] holds
     cols 0:64  = H[col(slot)] in bf16        (the gathered rows)
     cols 64:96 = val(slot) * onehot(dest_rel) (the scatter matrix S)
 - Device per chunk: PE matmul psum[64f, 32d] += G_chunk^T @ S_chunk,
   accumulated over the window's chunks (two windows share a psum tile
   via PE column halves).
 - Flush: one copy per window into acc[64, 12544] at the static offset
   32*w, alternating between the ACT and DVE engines.
 - Phase 2 (interleaved): after windows 4b..4b+3 flush, psum2[128d, 64f]
   = acc_blk^T @ W, relu on ACT, batched DMA out.
 - Host reassembles the full output by the row permutation.
"""
import sys

if "/opt/trn_rl_repo" not in sys.path:
    sys.path.insert(0, "/opt/trn_rl_repo")

import heapq

import numpy as np
import ml_dtypes

BF16 = np.dtype(ml_dtypes.bfloat16)

N_NODES = 100000
N_EDGES = 1600000
F = 64
NC = 8
DW = 24                         # dest-window width (rows per bin)
NWINS = 522                     # windows per core (522*24 = 12528)
NDEST = 98 * 128                # acc/out rows per core (12544)
NBLOCKS = 98                    # phase-2 128-row blocks (98*128 = 12544)
SLABW = 8                       # dest windows per DMA slab
BUFS = 12                       # slab buffers in flight


def _pack(a_rows, a_cols, a_vals):
    """LPT-pack dest rows into (core, window) bins; emit slot structure."""
    nbins = NC * NWINS
    counts = np.bincount(a_rows, minlength=N_NODES)
    order = np.argsort(-counts, kind="stable")
    # LPT with capacity: heaviest rows first into the lightest bin that
    # still has row space; edge capacity may overflow (rare, adds chunks)
    heap = [(0, 0, b) for b in range(nbins)]  # (load, nrows, bin)
    bin_rows = [[] for _ in range(nbins)]
    bin_load = np.zeros(nbins, np.int64)
    spill = []
    for r in order:
        c = int(counts[r])
        load, nrows, b = heap[0]
        if nrows + 1 >= DW:
            heapq.heappop(heap)  # bin full of rows, retire it
            spill.append((r, c))
            continue
        heapq.heapreplace(heap, (load + c, nrows + 1, b))
        bin_rows[b].append(r)
        bin_load[b] = load + c
    for r, c in spill:
        b = int(np.argmin(np.where(
            np.array([len(x) for x in bin_rows]) < DW, bin_load, 1 << 60)))
        bin_rows[b].append(r)
        bin_load[b] += c

    # deal bins to cores heaviest-first so window k has similar load on
    # every core (chunk counts are maxed across cores)
    bsort = np.argsort(-bin_load, kind="stable")
    row_core = np.empty(N_NODES, np.int32)
    row_local = np.empty(N_NODES, np.int32)
    binloads = np.zeros((NC, NWINS), np.int64)
    for i, b in enumerate(bsort):
        m, wdx = i % NC, i // NC
        binloads[m, wdx] = bin_load[b]
        rows = bin_rows[b]
        for k, r in enumerate(rows):
            row_core[r] = m
            row_local[r] = wdx * DW + k

    chunks_w = np.maximum((binloads.max(0) + 127) // 128, 1)  # [NWINS]
    wchunk = np.concatenate([[0], np.cumsum(chunks_w)])
    nchunks = int(wchunk[-1])
    nslots = 128 * nchunks

    ecore = row_core[a_rows]
    edest = row_local[a_rows].astype(np.int64)
    per_core = []
    for m in range(NC):
        sel = np.flatnonzero(ecore == m)
        dest = edest[sel]
        order2 = np.argsort(dest, kind="stable")
        dest = dest[order2]
        col = a_cols[sel].astype(np.int64)[order2]
        val = a_vals[sel][order2]
        w = dest // DW
        wcnt = np.bincount(w, minlength=NWINS)
        wstart = np.concatenate([[0], np.cumsum(wcnt)])
        slot = 128 * wchunk[w] + (np.arange(len(dest)) - wstart[w])
        slot_col = np.zeros(nslots, np.int64)
        slot_val = np.zeros(nslots, np.float32)
        slot_dr = np.zeros(nslots, np.int64)
        slot_col[slot] = col
        slot_val[slot] = val
        slot_dr[slot] = dest - DW * w
        per_core.append((slot_col, slot_val, slot_dr))

    structure = (tuple(int(c) for c in chunks_w), nchunks)
    return per_core, structure, row_core, row_local


def _expand(per_core, structure, H, W):
    """Build the interleaved G|S stream tiles from the slot structure."""
    _, nchunks = structure
    nslots = 128 * nchunks
    Hb = np.asarray(H, np.float32).astype(BF16)
    Wb = np.asarray(W, np.float32).astype(BF16)
    in_maps = []
    ar = np.arange(nslots)
    for slot_col, slot_val, slot_dr in per_core:
        GS = np.zeros((nchunks, 128, F + DW), BF16)
        GS[:, :, :F] = Hb[slot_col].reshape(nchunks, 128, F)
        S = np.zeros((nchunks, 128, DW), np.float32)
        S[ar // 128, ar % 128, slot_dr] = slot_val
        GS[:, :, F:] = S.astype(BF16)
        del S
        gs_tile = np.ascontiguousarray(
            GS.transpose(1, 0, 2).reshape(128, -1))
        del GS
        in_maps.append({"GS": gs_tile, "W": Wb})
    return in_maps


def _build(structure):
    import concourse.bass as bass  # noqa: F401
    import concourse.mybir as mybir
    import concourse.tile as tile
    from concourse import bacc
    from concourse.tile import ScopedClock

    class FixedTileContext(tile.TileContext):
        # This walrus build rejects >1 sync wait on the kernel-tail Drain;
        # split the waits across single-wait drains.
        def _drain_and_barrier(self, tick_clock, wait_clock):
            drain_inst = self.nc.sync.drain()
            wait_clock.add_sem_waits(
                drain_inst.ins, ScopedClock({None: tick_clock.global_clock})
            )
            si = drain_inst.ins.sync_info
            if si is not None and len(si.on_wait) > 1:
                waits = list(si.on_wait)
                drain_inst.ins.sync_info = mybir.SyncInfo(
                    on_wait=[waits[0]], on_update=list(si.on_update)
                )
                for wcond in waits[1:]:
                    d2 = self.nc.sync.drain()
                    d2.ins.sync_info = mybir.SyncInfo(on_wait=[wcond], on_update=[])
            self.nc.all_engine_barrier()
            assert self.sems is not None
            popped = self.nc._tile_sem_poison_stack.pop()
            assert popped is self._sem_poison
            self.nc.clear_and_free_semaphores(list(self.sems.allocated().values()))
            self.nc.all_engine_barrier()

    chunks_w, nchunks = structure
    wchunk = np.concatenate([[0], np.cumsum(chunks_w)])
    f32 = mybir.dt.float32
    bf16 = mybir.dt.bfloat16
    CW = F + DW

    nc = bacc.Bacc(None, target_bir_lowering=False)
    GSp = nc.declare_dram_parameter("GS", [128, nchunks * CW], bf16, isOutput=False)
    Wp = nc.declare_dram_parameter("W", [F, F], bf16, isOutput=False)
    out = nc.declare_dram_parameter("out", [NDEST, F], f32, isOutput=True)

    OBATCH = 7  # phase-2 output blocks per DMA (98 = 14*7)

    with FixedTileContext(nc) as tc:
        with (
            tc.tile_pool(name="const", bufs=1) as cpool,
            tc.tile_pool(name="gs", bufs=BUFS) as gspool,
            tc.tile_pool(name="psum", bufs=6, space="PSUM") as ppool,
            tc.tile_pool(name="psum2", bufs=2, space="PSUM") as p2pool,
            tc.tile_pool(name="outp", bufs=3) as opool,
        ):
            W_t = cpool.tile([F, F], bf16)
            acc = cpool.tile([F, NDEST], bf16)
            nc.sync.dma_start(out=W_t[:], in_=Wp[:])
            if NWINS * DW < NDEST:
                nc.vector.memset(acc[:, NWINS * DW:NDEST], 0.0)

            slabs = {}

            def fetch(sl):
                c0 = int(wchunk[sl * SLABW])
                c1 = int(wchunk[min((sl + 1) * SLABW, NWINS)])
                gs_t = gspool.tile([128, c1 - c0, CW], bf16)
                nc.sync.dma_start(
                    out=gs_t[:],
                    in_=GSp[:, c0 * CW:c1 * CW].rearrange(
                        "p (c x) -> p c x", x=CW),
                )
                slabs[sl] = (gs_t, c0)

            nslabs = (NWINS + SLABW - 1) // SLABW
            for sl in range(min(BUFS - 1, nslabs)):
                fetch(sl)

            o_t = [None]
            pending_b = [0]

            def emit_block(b):
                if b % OBATCH == 0:
                    o_t[0] = opool.tile([128, OBATCH, F], f32, name="o_t")
                psum_o = p2pool.tile([128, F], f32, space="PSUM")
                nc.tensor.matmul(
                    out=psum_o[:],
                    lhsT=acc[:, b * 128:(b + 1) * 128],
                    rhs=W_t[:],
                    start=True, stop=True,
                )
                nc.scalar.activation(
                    out=o_t[0][:, b % OBATCH, :], in_=psum_o[:],
                    func=mybir.ActivationFunctionType.Relu,
                )
                if b % OBATCH == OBATCH - 1:
                    ob = b // OBATCH
                    dst = out[ob * OBATCH * 128:(ob + 1) * OBATCH * 128, :]
                    nc.sync.dma_start(
                        out=dst.rearrange("(j p) f -> p j f", p=128),
                        in_=o_t[0][:],
                    )

            for t in range(NWINS // 2):
                w0 = 2 * t
                psum = ppool.tile([128, DW], f32, space="PSUM")
                nmax = max(chunks_w[w0], chunks_w[w0 + 1])
                for cc in range(nmax):
                    for j in (0, 1):
                        w = w0 + j
                        cw = chunks_w[w]
                        if cc >= cw:
                            continue
                        c = int(wchunk[w]) + cc
                        sl = w // SLABW
                        if sl not in slabs:
                            fetch(sl)
                        gs_t, c0 = slabs[sl]
                        nc.tensor.matmul(
                            out=psum[j * F:(j + 1) * F, :],
                            lhsT=gs_t[:, c - c0, 0:F],
                            rhs=gs_t[:, c - c0, F:CW],
                            start=(cc == 0),
                            stop=(cc == cw - 1),
                            tile_position=(0, j * F),
                        )
                for j in (0, 1):
                    w = w0 + j
                    eng = nc.scalar if w % 2 == 0 else None
                    if eng is not None:
                        nc.scalar.activation(
                            out=acc[:, w * DW:(w + 1) * DW],
                            in_=psum[j * F:(j + 1) * F, :],
                            func=mybir.ActivationFunctionType.Copy,
                        )
                    else:
                        nc.vector.tensor_copy(
                            out=acc[:, w * DW:(w + 1) * DW],
                            in_=psum[j * F:(j + 1) * F, :],
                        )
                if (w0 + 2) % SLABW == 0:
                    done_sl = w0 // SLABW
                    slabs.pop(done_sl, None)
                    nxt = done_sl + min(BUFS - 1, nslabs)
                    if nxt < nslabs and nxt not in slabs:
                        fetch(nxt)

                # phase 2 for any block whose windows have all flushed
                while (pending_b[0] < NBLOCKS
                       and (w0 + 2) * DW >= (pending_b[0] + 1) * 128):
                    emit_block(pending_b[0])
                    pending_b[0] += 1
            while pending_b[0] < NBLOCKS:
                emit_block(pending_b[0])
                pending_b[0] += 1

    nc.finalize()
    return nc


_cache = {}


def _get_nc(structure):
    if structure not in _cache:
        _cache[structure] = _build(structure)
    return _cache[structure]


def _run(in_maps, structure, trace=False, tmpdir=None):
    from concourse.bass_utils import run_bass_kernel_spmd
    nc = _get_nc(structure)
    return run_bass_kernel_spmd(
        nc, in_maps, list(range(NC)), trace=trace, tmpdir=tmpdir
    )


def _make_in_maps(a_rows, a_cols, a_vals, H, W):
    per_core, structure, row_core, row_local = _pack(
        np.asarray(a_rows), np.asarray(a_cols), np.asarray(a_vals)
    )
    in_maps = _expand(per_core, structure, H, W)
    return in_maps, structure, row_core, row_local


def kernel(a_rows, a_cols, a_vals, H, W, b):
    in_maps, structure, row_core, row_local = _make_in_maps(
        a_rows, a_cols, a_vals, H, W)
    res = _run(in_maps, structure)
    outs = [res.results[m]["out"] for m in range(NC)]
    out = np.empty((N_NODES, F), np.float32)
    for m in range(NC):
        rows = np.flatnonzero(row_core == m)
        out[rows] = outs[m][row_local[rows]]
    return out
